# revision 10
# baseline (speedup 1.0000x reference)
"""Distributed Trainium2 kernel for the fused attention-autoencoder layer.

Reference math (per head h):
  Q = x @ Wq_h^T + bq_h ; K = x @ Wk_h^T + bk_h ; V = x @ Wv_h^T + bv_h
  scores = K^T Q / sqrt(E); A = softmax(scores, -1); Zh = V @ A
  O = concat_h(Zh) @ Wz^T + bz ; LN1 = ln(O)*g1+b1 + x
  FN = LN1 @ Wf^T + bf ; out = ln(FN)*g2+b2 + LN1

Restructuring (head h lives on core h):
  With xa = [x | 1] (augmented) and G~ = xa^T xa (symmetric; computed
  distributed over S and AllReduced in 3 row-chunks):
    scores_h = Wka_h G~ Wqa_h^T / sqrt(E)  where Wka = [Wk|bk], Wqa = [Wq|bq]
  A_h = softmax(scores_h). Then instead of the S-sized Q/K/Zh matmuls:
    O_h = V_h A_h Wz_h^T = x (Wv_h^T A_h Wz_h^T) + 1 (bv_h^T A_h Wz_h^T)
  so each core does ONE S-sized matmul: Opart = x @ C_h + 1 r_h^T, with
    AT = A^T (PE transpose), B[e,o] = sum_f A[e,f] WzT[f,o], C = Wv^T B,
    r = bv^T B + bz/8 (bz pre-scaled on host so the RS sum restores bz).
  Cross-core: chunked bf16 ReduceScatter(sum_h) of Opart shards S;
  LN1/FFN/LN2 pipeline per chunk on S/8 rows per core; the host
  reassembles the (chunk-interleaved) shards.
  A tiny AllReduce "barrier" runs first so the per-core start skew is
  absorbed before the latency-critical G AllReduce.
"""

import numpy as np
import ml_dtypes

import concourse.bass as bass
import concourse.mybir as mybir
import concourse.tile as tile
from concourse import bacc
from concourse.bass_utils import run_bass_kernel_spmd
from concourse.masks import make_identity

S, E, H = 4096, 1024, 8
P = 128
EA = 1152          # augmented (E + ones col) padded to 9*128
NET = E // P       # 8
NAT = EA // P      # 9
SS = S // H        # 512 rows per core after reduce-scatter
NST = SS // P      # 4
NMT = S // P       # 32
NH = E // 512      # 2 free-dim halves
NCH = 4            # reduce-scatter chunks (each SS/NCH = 128 rows/core)
MPC = NMT // NCH   # Opart m-tiles per RS chunk (8)
NGC = 3            # G AllReduce chunks (3 row-tiles each)
KPC = NAT // NGC   # k-tiles per G chunk
EPS = 1e-5
SCALE = 1.0 / 32.0  # 1/sqrt(E)

F32 = mybir.dt.float32
BF16 = mybir.dt.bfloat16

# packed rows input: [bz/8, g1, b1, bf, g2, b2]; rows_bc holds the last 5
L_G1, L_B1, L_BF, L_G2, L_B2 = range(5)

LAST_RESULT = None  # test harness reads exec_time_ns off this


def _bcast_row(t: bass.AP) -> bass.AP:
    """[1, n] DRAM row -> partition-broadcast AP."""
    return bass.AP(tensor=t.tensor, offset=t.offset, ap=[[0, P], [1, t.shape[-1]]])


def build_nc():
    nc = bacc.Bacc(num_devices=H)

    xt = nc.declare_dram_parameter("xt", [E, S], BF16, isOutput=False)
    xsa = nc.declare_dram_parameter("xsa", [SS, EA], BF16, isOutput=False)
    xs = nc.declare_dram_parameter("xs", [SS, E], F32, isOutput=False)
    wqa = nc.declare_dram_parameter("wqa", [EA, E], BF16, isOutput=False)
    wka = nc.declare_dram_parameter("wka", [EA, E], BF16, isOutput=False)
    wv = nc.declare_dram_parameter("wv", [E, E], BF16, isOutput=False)
    wzT = nc.declare_dram_parameter("wzT", [E, E], BF16, isOutput=False)
    wfT = nc.declare_dram_parameter("wfT", [E, E], BF16, isOutput=False)
    bv = nc.declare_dram_parameter("bv", [P, NET], BF16, isOutput=False)
    rows = nc.declare_dram_parameter("rows", [6, E], F32, isOutput=False)
    out = nc.declare_dram_parameter("out", [SS, E], F32, isOutput=True)

    bar_in = nc.dram_tensor("bar_in", [1, 512], F32)
    bar_out = nc.dram_tensor("bar_out", [1, 512], F32, addr_space="Shared")
    g_part = nc.dram_tensor("g_part", [EA, EA], BF16)
    g_full = nc.dram_tensor("g_full", [EA, EA], BF16, addr_space="Shared")
    r_dram = nc.dram_tensor("r_dram", [1, E], F32)
    op_bounce = nc.dram_tensor("op_bounce", [S, E], BF16)
    rs_out = nc.dram_tensor("rs_out", [SS, E], BF16)

    rg = [list(range(H))]

    def mm_loop(lhs_fn, rhs_fn, nk, evac, ps_pool):
        for n in range(NH):
            ps = ps_pool.tile([P, 512], F32, tag="mm", name=f"psmm_{n}")
            for k in range(nk):
                nc.tensor.matmul(
                    ps, lhs_fn(k), rhs_fn(k, n), start=(k == 0), stop=(k == nk - 1)
                )
            evac(n, ps)

    with tile.TileContext(nc) as tc:
        with (
            tc.tile_pool(name="singles", bufs=1) as singles,
            tc.tile_pool(name="stat", bufs=4) as stat,
            tc.tile_pool(name="ps_mm", bufs=6, space="PSUM") as ps_mm,
            tc.tile_pool(name="ps_tr", bufs=2, space="PSUM") as ps_tr,
        ):
            # skew-absorbing warmup barrier: cheap AllReduce nobody reads
            nc.gpsimd.collective_compute(
                "AllReduce", mybir.AluOpType.add, replica_groups=rg,
                ins=[bar_in[:, :]], outs=[bar_out[:, :]],
            )
            ident = singles.tile([P, P], BF16)
            bz8_sb = singles.tile([1, E], F32)
            bv_sb = singles.tile([P, NET], BF16)
            rcp_sb = singles.tile([P, NET], F32)
            rbc_sb = singles.tile([P, E], F32)
            eps_sb = singles.tile([P, 1], F32)

            with tc.tile_pool(name="pc", bufs=1) as pc:
                c_sb = pc.tile([P, NET, E], BF16)
                with tc.tile_pool(name="pwz", bufs=1) as pwz:
                    wv_sb = pwz.tile([P, NET, E], BF16)
                    wzT_sb = pwz.tile([P, NET, E], BF16)
                    with tc.tile_pool(name="pb", bufs=1) as pb:
                        b_sb = pb.tile([P, NET, E], BF16)
                        with tc.tile_pool(name="pat", bufs=1) as pat:
                            at_sb = pat.tile([P, NET, E], BF16)
                            with tc.tile_pool(name="pwqk", bufs=1) as pwqk:
                                wqa_sb = pwqk.tile([P, NAT, E], BF16)
                                wka_sb = pwqk.tile([P, NAT, E], BF16)
                                u_sb = pwqk.tile([P, NAT, E], BF16)
                                with tc.tile_pool(name="pg", bufs=1) as pg, \
                                     tc.tile_pool(name="p1w", bufs=2) as p1w:
                                    # ===== phase 1: G~ partial + chunked AR =====
                                    xsa_sb = pg.tile([P, NST, EA], BF16)
                                    nc.sync.dma_start(
                                        out=xsa_sb,
                                        in_=xsa[:, :].rearrange("(t p) e -> p t e", p=P),
                                    )
                                    nchunks = [(0, 512), (512, 512), (1024, EA - 1024)]
                                    for ci in range(NGC):
                                        for mi in range(KPC):
                                            m = ci * KPC + mi
                                            gp = p1w.tile([P, EA], BF16, tag="gp")
                                            for (n0, nw) in nchunks:
                                                ps = ps_mm.tile(
                                                    [P, nw], F32, tag="mm", name="psg"
                                                )
                                                for k in range(NST):
                                                    nc.tensor.matmul(
                                                        ps,
                                                        xsa_sb[:, k, m * P : (m + 1) * P],
                                                        xsa_sb[:, k, n0 : n0 + nw],
                                                        start=(k == 0),
                                                        stop=(k == NST - 1),
                                                    )
                                                nc.vector.tensor_copy(
                                                    out=gp[:, n0 : n0 + nw], in_=ps
                                                )
                                            nc.sync.dma_start(
                                                out=g_part[m * P : (m + 1) * P, :], in_=gp
                                            )
                                        r0 = ci * KPC * P
                                        r1 = (ci + 1) * KPC * P
                                        nc.gpsimd.collective_compute(
                                            "AllReduce",
                                            mybir.AluOpType.add,
                                            replica_groups=rg,
                                            ins=[g_part[r0:r1, :]],
                                            outs=[g_full[r0:r1, :]],
                                        )

                                    # ---- constants / weights (emitted after
                                    # the collectives: G path wins DMA prio) ----
                                    make_identity(nc, ident)
                                    nc.sync.dma_start(out=bz8_sb, in_=rows[0:1, :])
                                    nc.sync.dma_start(out=bv_sb, in_=bv[:, :])
                                    nc.vector.memset(eps_sb, EPS)
                                    nc.sync.dma_start(
                                        out=wqa_sb,
                                        in_=wqa[:, :].rearrange("(t p) e -> p t e", p=P),
                                    )
                                    nc.sync.dma_start(
                                        out=wka_sb,
                                        in_=wka[:, :].rearrange("(t p) e -> p t e", p=P),
                                    )
                                    nc.sync.dma_start(
                                        out=wv_sb,
                                        in_=wv[:, :].rearrange("(t p) e -> p t e", p=P),
                                    )
                                    nc.sync.dma_start(
                                        out=wzT_sb,
                                        in_=wzT[:, :].rearrange("(t p) e -> p t e", p=P),
                                    )

                                    # ===== phase 2: U = G~ @ wqa, overlapping
                                    # the chunked AR (psum persists per chunk)
                                    g_sb = pg.tile([P, NAT, EA], BF16)
                                    for ci in range(NGC):
                                        nc.sync.dma_start(
                                            out=g_sb[:, ci * KPC : (ci + 1) * KPC, :],
                                            in_=g_full[
                                                ci * KPC * P : (ci + 1) * KPC * P, :
                                            ].rearrange("(t p) e -> p t e", p=P),
                                        )
                                    for (m0, m1) in [(0, 3), (3, 6), (6, 9)]:
                                        pss = {}
                                        for m in range(m0, m1):
                                            for n in range(NH):
                                                pss[m, n] = ps_mm.tile(
                                                    [P, 512], F32, tag="mm",
                                                    name=f"psu_{m}_{n}",
                                                )
                                        for ci in range(NGC):
                                            for m in range(m0, m1):
                                                for n in range(NH):
                                                    for kk in range(KPC):
                                                        k = ci * KPC + kk
                                                        nc.tensor.matmul(
                                                            pss[m, n],
                                                            g_sb[:, k, m * P : (m + 1) * P],
                                                            wqa_sb[:, k, n * 512 : (n + 1) * 512],
                                                            start=(k == 0),
                                                            stop=(k == NAT - 1),
                                                        )
                                        for m in range(m0, m1):
                                            for n in range(NH):
                                                nc.vector.tensor_copy(
                                                    out=u_sb[:, m, n * 512 : (n + 1) * 512],
                                                    in_=pss[m, n],
                                                )

                                # ===== phase 3: scores + softmax + A^T =====
                                with tc.tile_pool(name="p3", bufs=3) as p3:
                                    for m in range(NET):
                                        sc = p3.tile([P, E], F32, tag="sc")
                                        mm_loop(
                                            lambda k: wka_sb[:, k, m * P : (m + 1) * P],
                                            lambda k, n: u_sb[:, k, n * 512 : (n + 1) * 512],
                                            NAT,
                                            lambda n, ps: nc.vector.tensor_scalar_mul(
                                                sc[:, n * 512 : (n + 1) * 512], ps, SCALE
                                            ),
                                            ps_mm,
                                        )
                                        negmx = stat.tile([P, 1], F32, tag="negmx")
                                        nc.vector.reduce_max(
                                            out=negmx, in_=sc,
                                            axis=mybir.AxisListType.X, negate=True,
                                        )
                                        a_bf = p3.tile([P, E], BF16, tag="abf")
                                        rsum = stat.tile([P, 1], F32, tag="rsum")
                                        nc.scalar.activation(
                                            out=a_bf, in_=sc,
                                            func=mybir.ActivationFunctionType.Exp,
                                            bias=negmx, scale=1.0, accum_out=rsum,
                                        )
                                        nc.vector.reciprocal(
                                            out=rcp_sb[:, m : m + 1], in_=rsum
                                        )
                                        for fb in range(NET):
                                            pst = ps_tr.tile([P, P], BF16, tag="tr")
                                            nc.tensor.transpose(
                                                pst, a_bf[:, fb * P : (fb + 1) * P], ident
                                            )
                                            nc.vector.tensor_copy(
                                                out=at_sb[:, fb, m * P : (m + 1) * P],
                                                in_=pst,
                                            )

                            # ===== phase 4a: B = AT.T @ WzT (row-scaled) =====
                            for m in range(NET):
                                mm_loop(
                                    lambda k: at_sb[:, k, m * P : (m + 1) * P],
                                    lambda k, n: wzT_sb[:, k, n * 512 : (n + 1) * 512],
                                    NET,
                                    lambda n, ps: nc.vector.tensor_scalar_mul(
                                        b_sb[:, m, n * 512 : (n + 1) * 512],
                                        ps,
                                        rcp_sb[:, m : m + 1],
                                    ),
                                    ps_mm,
                                )

                        # ===== phase 4b: C = Wv^T B ; r = bv^T B + bz/8 =====
                        for m in range(NET):
                            mm_loop(
                                lambda k: wv_sb[:, k, m * P : (m + 1) * P],
                                lambda k, n: b_sb[:, k, n * 512 : (n + 1) * 512],
                                NET,
                                lambda n, ps: nc.vector.tensor_copy(
                                    out=c_sb[:, m, n * 512 : (n + 1) * 512], in_=ps
                                ),
                                ps_mm,
                            )
                        r_sb = stat.tile([1, E], F32, tag="rrow")
                        for n in range(NH):
                            psr = ps_mm.tile([1, 512], F32, tag="mm", name="psr")
                            for k in range(NET):
                                nc.tensor.matmul(
                                    psr,
                                    bv_sb[:, k : k + 1],
                                    b_sb[:, k, n * 512 : (n + 1) * 512],
                                    start=(k == 0),
                                    stop=(k == NET - 1),
                                )
                            nc.vector.tensor_add(
                                r_sb[:, n * 512 : (n + 1) * 512],
                                psr,
                                bz8_sb[:, n * 512 : (n + 1) * 512],
                            )
                        nc.sync.dma_start(out=r_dram[:, :], in_=r_sb)
                        nc.sync.dma_start(out=rbc_sb, in_=_bcast_row(r_dram[0:1, :]))

                # ===== phase 5: Opart + chunked RS; LN/FFN pipelined =====
                with tc.tile_pool(name="p5", bufs=3) as p5, \
                     tc.tile_pool(name="pln", bufs=1) as pln, \
                     tc.tile_pool(name="p7", bufs=2) as p7:
                    wfT_sb = pln.tile([P, NET, E], BF16)
                    nc.sync.dma_start(
                        out=wfT_sb, in_=wfT[:, :].rearrange("(t p) e -> p t e", p=P)
                    )
                    rows_bc = pln.tile([P, 5, E], F32)
                    for k in range(5):
                        nc.sync.dma_start(
                            out=rows_bc[:, k, :], in_=_bcast_row(rows[k + 1 : k + 2, :])
                        )
                    ln1_sb = pln.tile([P, NST, E], F32)
                    l1t_sb = pln.tile([P, NET, SS], BF16)
                    xt_re = xt[:, :].rearrange("(t p) s -> p t s", p=P)

                    def layer_norm(dst, src, r_g, r_b):
                        bst = stat.tile([P, 2, 6], F32, tag="bst")
                        nc.vector.bn_stats(out=bst[:, 0, :], in_=src[:, 0:512])
                        nc.vector.bn_stats(out=bst[:, 1, :], in_=src[:, 512:E])
                        mv = stat.tile([P, 2], F32, tag="mv")
                        nc.vector.bn_aggr(out=mv, in_=bst)
                        sd = stat.tile([P, 1], F32, tag="sd")
                        nc.scalar.activation(
                            out=sd, in_=mv[:, 1:2],
                            func=mybir.ActivationFunctionType.Sqrt, bias=eps_sb[:, :],
                        )
                        rstd = stat.tile([P, 1], F32, tag="rstd")
                        nc.vector.reciprocal(out=rstd, in_=sd)
                        nc.vector.tensor_scalar(
                            out=dst, in0=src, scalar1=mv[:, 0:1], scalar2=rstd,
                            op0=mybir.AluOpType.subtract, op1=mybir.AluOpType.mult,
                        )
                        nc.vector.tensor_mul(dst, dst, rows_bc[:, r_g, :])
                        nc.vector.tensor_add(dst, dst, rows_bc[:, r_b, :])

                    for c in range(NCH):
                        for mi in range(MPC):
                            m = c * MPC + mi
                            xtc = p5.tile([P, NET, P], BF16, tag="xtc")
                            nc.sync.dma_start(
                                out=xtc, in_=xt_re[:, :, m * P : (m + 1) * P]
                            )
                            o_sb = p5.tile([P, E], BF16, tag="osb")
                            mm_loop(
                                lambda k: xtc[:, k, :],
                                lambda k, n: c_sb[:, k, n * 512 : (n + 1) * 512],
                                NET,
                                lambda n, ps: nc.vector.tensor_add(
                                    o_sb[:, n * 512 : (n + 1) * 512],
                                    ps,
                                    rbc_sb[:, n * 512 : (n + 1) * 512],
                                ),
                                ps_mm,
                            )
                            nc.sync.dma_start(
                                out=op_bounce[m * P : (m + 1) * P, :], in_=o_sb
                            )
                        nc.gpsimd.collective_compute(
                            "ReduceScatter",
                            mybir.AluOpType.add,
                            replica_groups=rg,
                            ins=[op_bounce[c * MPC * P : (c + 1) * MPC * P, :]],
                            outs=[rs_out[c * P : (c + 1) * P, :]],
                        )

                        # LN1 + FFN + LN2 for this chunk (overlaps the next
                        # chunk's Opart compute and RS)
                        st = c
                        t1 = ln1_sb[:, st, :]
                        ot = p7.tile([P, E], BF16, tag="ot")
                        nc.sync.dma_start(out=ot, in_=rs_out[st * P : (st + 1) * P, :])
                        ln = p7.tile([P, E], F32, tag="ln")
                        layer_norm(ln, ot, L_G1, L_B1)
                        xst = p7.tile([P, E], F32, tag="xst")
                        nc.sync.dma_start(out=xst, in_=xs[st * P : (st + 1) * P, :])
                        nc.vector.tensor_add(t1, ln, xst)
                        lbf = p7.tile([P, E], BF16, tag="lbf")
                        nc.vector.tensor_copy(out=lbf, in_=t1)
                        for eb in range(NET):
                            pst = ps_tr.tile([P, P], BF16, tag="tr")
                            nc.tensor.transpose(pst, lbf[:, eb * P : (eb + 1) * P], ident)
                            nc.vector.tensor_copy(
                                out=l1t_sb[:, eb, st * P : (st + 1) * P], in_=pst
                            )
                        f1 = p7.tile([P, E], F32, tag="f1")
                        mm_loop(
                            lambda k: l1t_sb[:, k, st * P : (st + 1) * P],
                            lambda k, n: wfT_sb[:, k, n * 512 : (n + 1) * 512],
                            NET,
                            lambda n, ps: nc.vector.tensor_add(
                                f1[:, n * 512 : (n + 1) * 512],
                                ps,
                                rows_bc[:, L_BF, n * 512 : (n + 1) * 512],
                            ),
                            ps_mm,
                        )
                        ln2 = p7.tile([P, E], F32, tag="ln2")
                        layer_norm(ln2, f1, L_G2, L_B2)
                        fo = p7.tile([P, E], F32, tag="ln")
                        nc.vector.tensor_add(fo, ln2, ln1_sb[:, st, :])
                        nc.sync.dma_start(out=out[st * P : (st + 1) * P, :], in_=fo)

    nc.finalize()
    return nc


_NC_CACHE = None


def _shard_rows(h):
    """Global S-rows owned by core h (RS chunk layout)."""
    idx = []
    for c in range(NCH):
        base = c * (S // NCH) + h * P
        idx.extend(range(base, base + P))
    return np.array(idx)


def kernel(**inputs) -> np.ndarray:
    global _NC_CACHE, LAST_RESULT
    x = np.asarray(inputs["x"], np.float32)
    Wq = np.asarray(inputs["Wq"], np.float32)
    bq = np.asarray(inputs["bq"], np.float32)
    Wk = np.asarray(inputs["Wk"], np.float32)
    bk = np.asarray(inputs["bk"], np.float32)
    Wv = np.asarray(inputs["Wv"], np.float32)
    bv = np.asarray(inputs["bv"], np.float32)
    Wz = np.asarray(inputs["Wz"], np.float32)
    bz = np.asarray(inputs["bz"], np.float32)
    g1 = np.asarray(inputs["g1"], np.float32)
    b1 = np.asarray(inputs["b1"], np.float32)
    Wf = np.asarray(inputs["Wf"], np.float32)
    bf_ = np.asarray(inputs["bf"], np.float32)
    g2 = np.asarray(inputs["g2"], np.float32)
    b2 = np.asarray(inputs["b2"], np.float32)

    BF = ml_dtypes.bfloat16
    if _NC_CACHE is None:
        _NC_CACHE = build_nc()
    nc = _NC_CACHE

    xt_np = np.ascontiguousarray(x.T).astype(BF)
    wfT_np = np.ascontiguousarray(Wf.T).astype(BF)
    rows_np = np.ascontiguousarray(
        np.stack([bz / H, g1, b1, bf_, g2, b2]).astype(np.float32)
    )
    pad_w = np.zeros((EA - E - 1, E), np.float32)

    in_maps = []
    for h in range(H):
        gsl = slice(h * SS, (h + 1) * SS)  # contiguous shard for G partial
        xga = x[gsl]
        xsa_h = np.concatenate(
            [xga, np.ones((SS, 1), np.float32), np.zeros((SS, EA - E - 1), np.float32)],
            axis=1,
        ).astype(BF)
        xs_h = np.ascontiguousarray(x[_shard_rows(h)])  # residual rows (RS layout)
        wqa_h = np.concatenate([Wq[h].T, bq[h][None, :], pad_w], axis=0).astype(BF)
        wka_h = np.concatenate([Wk[h].T, bk[h][None, :], pad_w], axis=0).astype(BF)
        wzT_h = np.ascontiguousarray(Wz[:, h * E : (h + 1) * E].T).astype(BF)
        bv_h = np.ascontiguousarray(bv[h].reshape(NET, P).T).astype(BF)
        in_maps.append(
            {
                "xt": xt_np,
                "xsa": np.ascontiguousarray(xsa_h),
                "xs": xs_h,
                "wqa": np.ascontiguousarray(wqa_h),
                "wka": np.ascontiguousarray(wka_h),
                "wv": Wv[h].astype(BF),
                "wzT": wzT_h,
                "wfT": wfT_np,
                "bv": bv_h,
                "rows": rows_np,
            }
        )

    res = run_bass_kernel_spmd(nc, in_maps, list(range(H)))
    LAST_RESULT = res
    out = np.empty((S, E), np.float32)
    for h in range(H):
        out[_shard_rows(h)] = res.results[h]["out"]
    return out


# revision 11
# speedup vs baseline: 1.2974x; 1.2974x over previous
"""Distributed Trainium2 kernel for the fused attention-autoencoder layer.

Reference math (per head h):
  Q = x @ Wq_h^T + bq_h ; K = x @ Wk_h^T + bk_h ; V = x @ Wv_h^T + bv_h
  scores = K^T Q / sqrt(E); A = softmax(scores, -1); Zh = V @ A
  O = concat_h(Zh) @ Wz^T + bz ; LN1 = ln(O)*g1+b1 + x
  FN = LN1 @ Wf^T + bf ; out = ln(FN)*g2+b2 + LN1

Restructuring (head h lives on core h):
  With xa = [x | 1] (augmented) and G~ = xa^T xa (symmetric; computed
  distributed over S and AllReduced in 3 row-chunks):
    scores_h = Wka_h G~ Wqa_h^T / sqrt(E)  where Wka = [Wk|bk], Wqa = [Wq|bq]
  A_h = softmax(scores_h). Then instead of the S-sized Q/K/Zh matmuls:
    O_h = V_h A_h Wz_h^T = x (Wv_h^T A_h Wz_h^T) + 1 (bv_h^T A_h Wz_h^T)
  so each core does ONE S-sized matmul: Opart = x @ C_h + 1 r_h^T, with
    AT = A^T (PE transpose), B[e,o] = sum_f A[e,f] WzT[f,o], C = Wv^T B,
    r = bv^T B + bz/8 (bz pre-scaled on host so the RS sum restores bz).
  Cross-core: chunked bf16 ReduceScatter(sum_h) of Opart shards S;
  LN1/FFN/LN2 pipeline per chunk on S/8 rows per core; the host
  reassembles the (chunk-interleaved) shards.
  A tiny AllReduce "barrier" runs first so the per-core start skew is
  absorbed before the latency-critical G AllReduce.
"""

import numpy as np
import ml_dtypes

import concourse.bass as bass
import concourse.mybir as mybir
import concourse.tile as tile
from concourse import bacc
from concourse.bass_utils import run_bass_kernel_spmd
from concourse.masks import make_identity

S, E, H = 4096, 1024, 8
P = 128
EA = 1152          # augmented (E + ones col) padded to 9*128
NET = E // P       # 8
NAT = EA // P      # 9
SS = S // H        # 512 rows per core after reduce-scatter
NST = SS // P      # 4
NMT = S // P       # 32
NH = E // 512      # 2 free-dim halves
NCH = 4            # reduce-scatter chunks (each SS/NCH = 128 rows/core)
MPC = NMT // NCH   # Opart m-tiles per RS chunk (8)
NGC = 3            # G AllReduce chunks (3 row-tiles each)
KPC = NAT // NGC   # k-tiles per G chunk
EPS = 1e-5
SCALE = 1.0 / 32.0  # 1/sqrt(E)

F32 = mybir.dt.float32
BF16 = mybir.dt.bfloat16

# packed rows input: [bz/8, g1, b1, bf, g2, b2]; rows_bc holds the last 5
L_G1, L_B1, L_BF, L_G2, L_B2 = range(5)

LAST_RESULT = None  # test harness reads exec_time_ns off this


def _bcast_row(t: bass.AP) -> bass.AP:
    """[1, n] DRAM row -> partition-broadcast AP."""
    return bass.AP(tensor=t.tensor, offset=t.offset, ap=[[0, P], [1, t.shape[-1]]])


def build_nc():
    nc = bacc.Bacc(num_devices=H)

    xt = nc.declare_dram_parameter("xt", [E, S], BF16, isOutput=False)
    xsa = nc.declare_dram_parameter("xsa", [SS, EA], BF16, isOutput=False)
    xs = nc.declare_dram_parameter("xs", [SS, E], F32, isOutput=False)
    wqa = nc.declare_dram_parameter("wqa", [EA, E], BF16, isOutput=False)
    wka = nc.declare_dram_parameter("wka", [EA, E], BF16, isOutput=False)
    wv = nc.declare_dram_parameter("wv", [E, E], BF16, isOutput=False)
    wzT = nc.declare_dram_parameter("wzT", [E, E], BF16, isOutput=False)
    wfT = nc.declare_dram_parameter("wfT", [E, E], BF16, isOutput=False)
    bv = nc.declare_dram_parameter("bv", [P, NET], BF16, isOutput=False)
    rows = nc.declare_dram_parameter("rows", [6, E], F32, isOutput=False)
    out = nc.declare_dram_parameter("out", [SS, E], F32, isOutput=True)

    bar_in = nc.dram_tensor("bar_in", [1, 512], F32)
    bar_out = nc.dram_tensor("bar_out", [1, 512], F32, addr_space="Shared")
    g_part = nc.dram_tensor("g_part", [EA, EA], BF16)
    g_full = nc.dram_tensor("g_full", [EA, EA], BF16, addr_space="Shared")
    r_dram = nc.dram_tensor("r_dram", [1, E], F32)
    op_bounce = nc.dram_tensor("op_bounce", [S, E], BF16)
    rs_out = nc.dram_tensor("rs_out", [SS, E], BF16)

    rg = [list(range(H))]

    def mm_loop(lhs_fn, rhs_fn, nk, evac, ps_pool):
        for n in range(NH):
            ps = ps_pool.tile([P, 512], F32, tag="mm", name=f"psmm_{n}")
            for k in range(nk):
                nc.tensor.matmul(
                    ps, lhs_fn(k), rhs_fn(k, n), start=(k == 0), stop=(k == nk - 1)
                )
            evac(n, ps)

    with tile.TileContext(nc) as tc:
        with (
            tc.tile_pool(name="singles", bufs=1) as singles,
            tc.tile_pool(name="stat", bufs=4) as stat,
            tc.tile_pool(name="ps_mm", bufs=6, space="PSUM") as ps_mm,
            tc.tile_pool(name="ps_tr", bufs=2, space="PSUM") as ps_tr,
        ):
            ident = singles.tile([P, P], BF16)
            bz8_sb = singles.tile([1, E], F32)
            bv_sb = singles.tile([P, NET], BF16)
            rcp_sb = singles.tile([P, NET], F32)
            rbc_sb = singles.tile([P, E], F32)
            eps_sb = singles.tile([P, 1], F32)

            with tc.tile_pool(name="pc", bufs=1) as pc:
                c_sb = pc.tile([P, NET, E], BF16)
                with tc.tile_pool(name="pwz", bufs=1) as pwz:
                    wv_sb = pwz.tile([P, NET, E], BF16)
                    wzT_sb = pwz.tile([P, NET, E], BF16)
                    with tc.tile_pool(name="pb", bufs=1) as pb:
                        b_sb = pb.tile([P, NET, E], BF16)
                        with tc.tile_pool(name="pat", bufs=1) as pat:
                            at_sb = pat.tile([P, NET, E], BF16)
                            with tc.tile_pool(name="pwqk", bufs=1) as pwqk:
                                wqa_sb = pwqk.tile([P, NAT, E], BF16)
                                wka_sb = pwqk.tile([P, NAT, E], BF16)
                                u_sb = pwqk.tile([P, NAT, E], BF16)
                                with tc.tile_pool(name="pg", bufs=1) as pg, \
                                     tc.tile_pool(name="p1w", bufs=2) as p1w:
                                    # ===== phase 1: G~ partial + chunked AR =====
                                    xsa_sb = pg.tile([P, NST, EA], BF16)
                                    nc.sync.dma_start(
                                        out=xsa_sb,
                                        in_=xsa[:, :].rearrange("(t p) e -> p t e", p=P),
                                    )
                                    nchunks = [(0, 512), (512, 512), (1024, EA - 1024)]
                                    for ci in range(NGC):
                                        for mi in range(KPC):
                                            m = ci * KPC + mi
                                            gp = p1w.tile([P, EA], BF16, tag="gp")
                                            for (n0, nw) in nchunks:
                                                ps = ps_mm.tile(
                                                    [P, nw], F32, tag="mm", name="psg"
                                                )
                                                for k in range(NST):
                                                    nc.tensor.matmul(
                                                        ps,
                                                        xsa_sb[:, k, m * P : (m + 1) * P],
                                                        xsa_sb[:, k, n0 : n0 + nw],
                                                        start=(k == 0),
                                                        stop=(k == NST - 1),
                                                    )
                                                nc.vector.tensor_copy(
                                                    out=gp[:, n0 : n0 + nw], in_=ps
                                                )
                                            nc.sync.dma_start(
                                                out=g_part[m * P : (m + 1) * P, :], in_=gp
                                            )
                                        r0 = ci * KPC * P
                                        r1 = (ci + 1) * KPC * P
                                        nc.gpsimd.collective_compute(
                                            "AllReduce",
                                            mybir.AluOpType.add,
                                            replica_groups=rg,
                                            ins=[g_part[r0:r1, :]],
                                            outs=[g_full[r0:r1, :]],
                                        )

                                    # ---- constants / weights (emitted after
                                    # the collectives: G path wins DMA prio) ----
                                    make_identity(nc, ident)
                                    nc.sync.dma_start(out=bz8_sb, in_=rows[0:1, :])
                                    nc.sync.dma_start(out=bv_sb, in_=bv[:, :])
                                    nc.vector.memset(eps_sb, EPS)
                                    nc.sync.dma_start(
                                        out=wqa_sb,
                                        in_=wqa[:, :].rearrange("(t p) e -> p t e", p=P),
                                    )
                                    nc.sync.dma_start(
                                        out=wka_sb,
                                        in_=wka[:, :].rearrange("(t p) e -> p t e", p=P),
                                    )
                                    nc.sync.dma_start(
                                        out=wv_sb,
                                        in_=wv[:, :].rearrange("(t p) e -> p t e", p=P),
                                    )
                                    nc.sync.dma_start(
                                        out=wzT_sb,
                                        in_=wzT[:, :].rearrange("(t p) e -> p t e", p=P),
                                    )

                                    # ===== phase 2: U = G~ @ wqa, overlapping
                                    # the chunked AR (psum persists per chunk)
                                    g_sb = pg.tile([P, NAT, EA], BF16)
                                    for ci in range(NGC):
                                        nc.sync.dma_start(
                                            out=g_sb[:, ci * KPC : (ci + 1) * KPC, :],
                                            in_=g_full[
                                                ci * KPC * P : (ci + 1) * KPC * P, :
                                            ].rearrange("(t p) e -> p t e", p=P),
                                        )
                                    for (m0, m1) in [(0, 3), (3, 6), (6, 9)]:
                                        pss = {}
                                        for m in range(m0, m1):
                                            for n in range(NH):
                                                pss[m, n] = ps_mm.tile(
                                                    [P, 512], F32, tag="mm",
                                                    name=f"psu_{m}_{n}",
                                                )
                                        for ci in range(NGC):
                                            for m in range(m0, m1):
                                                for n in range(NH):
                                                    for kk in range(KPC):
                                                        k = ci * KPC + kk
                                                        nc.tensor.matmul(
                                                            pss[m, n],
                                                            g_sb[:, k, m * P : (m + 1) * P],
                                                            wqa_sb[:, k, n * 512 : (n + 1) * 512],
                                                            start=(k == 0),
                                                            stop=(k == NAT - 1),
                                                        )
                                        for m in range(m0, m1):
                                            for n in range(NH):
                                                nc.vector.tensor_copy(
                                                    out=u_sb[:, m, n * 512 : (n + 1) * 512],
                                                    in_=pss[m, n],
                                                )

                                # ===== phase 3: scores + softmax + A^T =====
                                with tc.tile_pool(name="p3", bufs=3) as p3:
                                    for m in range(NET):
                                        sc = p3.tile([P, E], F32, tag="sc")
                                        mm_loop(
                                            lambda k: wka_sb[:, k, m * P : (m + 1) * P],
                                            lambda k, n: u_sb[:, k, n * 512 : (n + 1) * 512],
                                            NAT,
                                            lambda n, ps: nc.vector.tensor_scalar_mul(
                                                sc[:, n * 512 : (n + 1) * 512], ps, SCALE
                                            ),
                                            ps_mm,
                                        )
                                        negmx = stat.tile([P, 1], F32, tag="negmx")
                                        nc.vector.reduce_max(
                                            out=negmx, in_=sc,
                                            axis=mybir.AxisListType.X, negate=True,
                                        )
                                        a_bf = p3.tile([P, E], BF16, tag="abf")
                                        rsum = stat.tile([P, 1], F32, tag="rsum")
                                        nc.scalar.activation(
                                            out=a_bf, in_=sc,
                                            func=mybir.ActivationFunctionType.Exp,
                                            bias=negmx, scale=1.0, accum_out=rsum,
                                        )
                                        nc.vector.reciprocal(
                                            out=rcp_sb[:, m : m + 1], in_=rsum
                                        )
                                        for fb in range(NET):
                                            pst = ps_tr.tile([P, P], BF16, tag="tr")
                                            nc.tensor.transpose(
                                                pst, a_bf[:, fb * P : (fb + 1) * P], ident
                                            )
                                            nc.vector.tensor_copy(
                                                out=at_sb[:, fb, m * P : (m + 1) * P],
                                                in_=pst,
                                            )

                            # ===== phase 4a: B = AT.T @ WzT (row-scaled) =====
                            for m in range(NET):
                                mm_loop(
                                    lambda k: at_sb[:, k, m * P : (m + 1) * P],
                                    lambda k, n: wzT_sb[:, k, n * 512 : (n + 1) * 512],
                                    NET,
                                    lambda n, ps: nc.vector.tensor_scalar_mul(
                                        b_sb[:, m, n * 512 : (n + 1) * 512],
                                        ps,
                                        rcp_sb[:, m : m + 1],
                                    ),
                                    ps_mm,
                                )

                        # ===== phase 4b: C = Wv^T B ; r = bv^T B + bz/8 =====
                        for m in range(NET):
                            mm_loop(
                                lambda k: wv_sb[:, k, m * P : (m + 1) * P],
                                lambda k, n: b_sb[:, k, n * 512 : (n + 1) * 512],
                                NET,
                                lambda n, ps: nc.vector.tensor_copy(
                                    out=c_sb[:, m, n * 512 : (n + 1) * 512], in_=ps
                                ),
                                ps_mm,
                            )
                        r_sb = stat.tile([1, E], F32, tag="rrow")
                        for n in range(NH):
                            psr = ps_mm.tile([1, 512], F32, tag="mm", name="psr")
                            for k in range(NET):
                                nc.tensor.matmul(
                                    psr,
                                    bv_sb[:, k : k + 1],
                                    b_sb[:, k, n * 512 : (n + 1) * 512],
                                    start=(k == 0),
                                    stop=(k == NET - 1),
                                )
                            nc.vector.tensor_add(
                                r_sb[:, n * 512 : (n + 1) * 512],
                                psr,
                                bz8_sb[:, n * 512 : (n + 1) * 512],
                            )
                        nc.sync.dma_start(out=r_dram[:, :], in_=r_sb)
                        nc.sync.dma_start(out=rbc_sb, in_=_bcast_row(r_dram[0:1, :]))

                # ===== phase 5: Opart + chunked RS; LN/FFN pipelined =====
                with tc.tile_pool(name="p5", bufs=3) as p5, \
                     tc.tile_pool(name="pln", bufs=1) as pln, \
                     tc.tile_pool(name="p7", bufs=2) as p7:
                    wfT_sb = pln.tile([P, NET, E], BF16)
                    nc.sync.dma_start(
                        out=wfT_sb, in_=wfT[:, :].rearrange("(t p) e -> p t e", p=P)
                    )
                    rows_bc = pln.tile([P, 5, E], F32)
                    for k in range(5):
                        nc.sync.dma_start(
                            out=rows_bc[:, k, :], in_=_bcast_row(rows[k + 1 : k + 2, :])
                        )
                    ln1_sb = pln.tile([P, NST, E], F32)
                    l1t_sb = pln.tile([P, NET, SS], BF16)
                    xt_re = xt[:, :].rearrange("(t p) s -> p t s", p=P)

                    def layer_norm(dst, src, r_g, r_b):
                        bst = stat.tile([P, 2, 6], F32, tag="bst")
                        nc.vector.bn_stats(out=bst[:, 0, :], in_=src[:, 0:512])
                        nc.vector.bn_stats(out=bst[:, 1, :], in_=src[:, 512:E])
                        mv = stat.tile([P, 2], F32, tag="mv")
                        nc.vector.bn_aggr(out=mv, in_=bst)
                        sd = stat.tile([P, 1], F32, tag="sd")
                        nc.scalar.activation(
                            out=sd, in_=mv[:, 1:2],
                            func=mybir.ActivationFunctionType.Sqrt, bias=eps_sb[:, :],
                        )
                        rstd = stat.tile([P, 1], F32, tag="rstd")
                        nc.vector.reciprocal(out=rstd, in_=sd)
                        nc.vector.tensor_scalar(
                            out=dst, in0=src, scalar1=mv[:, 0:1], scalar2=rstd,
                            op0=mybir.AluOpType.subtract, op1=mybir.AluOpType.mult,
                        )
                        nc.vector.tensor_mul(dst, dst, rows_bc[:, r_g, :])
                        nc.vector.tensor_add(dst, dst, rows_bc[:, r_b, :])

                    for c in range(NCH):
                        for mi in range(MPC):
                            m = c * MPC + mi
                            xtc = p5.tile([P, NET, P], BF16, tag="xtc")
                            nc.sync.dma_start(
                                out=xtc, in_=xt_re[:, :, m * P : (m + 1) * P]
                            )
                            o_sb = p5.tile([P, E], BF16, tag="osb")
                            mm_loop(
                                lambda k: xtc[:, k, :],
                                lambda k, n: c_sb[:, k, n * 512 : (n + 1) * 512],
                                NET,
                                lambda n, ps: nc.vector.tensor_add(
                                    o_sb[:, n * 512 : (n + 1) * 512],
                                    ps,
                                    rbc_sb[:, n * 512 : (n + 1) * 512],
                                ),
                                ps_mm,
                            )
                            nc.sync.dma_start(
                                out=op_bounce[m * P : (m + 1) * P, :], in_=o_sb
                            )
                        nc.gpsimd.collective_compute(
                            "ReduceScatter",
                            mybir.AluOpType.add,
                            replica_groups=rg,
                            ins=[op_bounce[c * MPC * P : (c + 1) * MPC * P, :]],
                            outs=[rs_out[c * P : (c + 1) * P, :]],
                        )

                    # LN1 + FFN + LN2 per chunk, after all Opart matmuls so
                    # the in-order PE/DVE queues never stall on an RS wait
                    for st in range(NCH):
                        t1 = ln1_sb[:, st, :]
                        ot = p7.tile([P, E], BF16, tag="ot")
                        nc.sync.dma_start(out=ot, in_=rs_out[st * P : (st + 1) * P, :])
                        ln = p7.tile([P, E], F32, tag="ln")
                        layer_norm(ln, ot, L_G1, L_B1)
                        xst = p7.tile([P, E], F32, tag="xst")
                        nc.sync.dma_start(out=xst, in_=xs[st * P : (st + 1) * P, :])
                        nc.vector.tensor_add(t1, ln, xst)
                        lbf = p7.tile([P, E], BF16, tag="lbf")
                        nc.vector.tensor_copy(out=lbf, in_=t1)
                        for eb in range(NET):
                            pst = ps_tr.tile([P, P], BF16, tag="tr")
                            nc.tensor.transpose(pst, lbf[:, eb * P : (eb + 1) * P], ident)
                            nc.vector.tensor_copy(
                                out=l1t_sb[:, eb, st * P : (st + 1) * P], in_=pst
                            )
                        f1 = p7.tile([P, E], F32, tag="f1")
                        mm_loop(
                            lambda k: l1t_sb[:, k, st * P : (st + 1) * P],
                            lambda k, n: wfT_sb[:, k, n * 512 : (n + 1) * 512],
                            NET,
                            lambda n, ps: nc.vector.tensor_add(
                                f1[:, n * 512 : (n + 1) * 512],
                                ps,
                                rows_bc[:, L_BF, n * 512 : (n + 1) * 512],
                            ),
                            ps_mm,
                        )
                        ln2 = p7.tile([P, E], F32, tag="ln2")
                        layer_norm(ln2, f1, L_G2, L_B2)
                        fo = p7.tile([P, E], F32, tag="ln")
                        nc.vector.tensor_add(fo, ln2, ln1_sb[:, st, :])
                        nc.sync.dma_start(out=out[st * P : (st + 1) * P, :], in_=fo)

    nc.finalize()
    return nc


_NC_CACHE = None


def _shard_rows(h):
    """Global S-rows owned by core h (RS chunk layout)."""
    idx = []
    for c in range(NCH):
        base = c * (S // NCH) + h * P
        idx.extend(range(base, base + P))
    return np.array(idx)


def kernel(**inputs) -> np.ndarray:
    global _NC_CACHE, LAST_RESULT
    x = np.asarray(inputs["x"], np.float32)
    Wq = np.asarray(inputs["Wq"], np.float32)
    bq = np.asarray(inputs["bq"], np.float32)
    Wk = np.asarray(inputs["Wk"], np.float32)
    bk = np.asarray(inputs["bk"], np.float32)
    Wv = np.asarray(inputs["Wv"], np.float32)
    bv = np.asarray(inputs["bv"], np.float32)
    Wz = np.asarray(inputs["Wz"], np.float32)
    bz = np.asarray(inputs["bz"], np.float32)
    g1 = np.asarray(inputs["g1"], np.float32)
    b1 = np.asarray(inputs["b1"], np.float32)
    Wf = np.asarray(inputs["Wf"], np.float32)
    bf_ = np.asarray(inputs["bf"], np.float32)
    g2 = np.asarray(inputs["g2"], np.float32)
    b2 = np.asarray(inputs["b2"], np.float32)

    BF = ml_dtypes.bfloat16
    if _NC_CACHE is None:
        _NC_CACHE = build_nc()
    nc = _NC_CACHE

    xt_np = np.ascontiguousarray(x.T).astype(BF)
    wfT_np = np.ascontiguousarray(Wf.T).astype(BF)
    rows_np = np.ascontiguousarray(
        np.stack([bz / H, g1, b1, bf_, g2, b2]).astype(np.float32)
    )
    pad_w = np.zeros((EA - E - 1, E), np.float32)

    in_maps = []
    for h in range(H):
        gsl = slice(h * SS, (h + 1) * SS)  # contiguous shard for G partial
        xga = x[gsl]
        xsa_h = np.concatenate(
            [xga, np.ones((SS, 1), np.float32), np.zeros((SS, EA - E - 1), np.float32)],
            axis=1,
        ).astype(BF)
        xs_h = np.ascontiguousarray(x[_shard_rows(h)])  # residual rows (RS layout)
        wqa_h = np.concatenate([Wq[h].T, bq[h][None, :], pad_w], axis=0).astype(BF)
        wka_h = np.concatenate([Wk[h].T, bk[h][None, :], pad_w], axis=0).astype(BF)
        wzT_h = np.ascontiguousarray(Wz[:, h * E : (h + 1) * E].T).astype(BF)
        bv_h = np.ascontiguousarray(bv[h].reshape(NET, P).T).astype(BF)
        in_maps.append(
            {
                "xt": xt_np,
                "xsa": np.ascontiguousarray(xsa_h),
                "xs": xs_h,
                "wqa": np.ascontiguousarray(wqa_h),
                "wka": np.ascontiguousarray(wka_h),
                "wv": Wv[h].astype(BF),
                "wzT": wzT_h,
                "wfT": wfT_np,
                "bv": bv_h,
                "rows": rows_np,
            }
        )

    res = run_bass_kernel_spmd(nc, in_maps, list(range(H)))
    LAST_RESULT = res
    out = np.empty((S, E), np.float32)
    for h in range(H):
        out[_shard_rows(h)] = res.results[h]["out"]
    return out


# revision 12
# speedup vs baseline: 1.3271x; 1.0229x over previous
"""Distributed Trainium2 kernel for the fused attention-autoencoder layer.

Reference math (per head h):
  Q = x @ Wq_h^T + bq_h ; K = x @ Wk_h^T + bk_h ; V = x @ Wv_h^T + bv_h
  scores = K^T Q / sqrt(E); A = softmax(scores, -1); Zh = V @ A
  O = concat_h(Zh) @ Wz^T + bz ; LN1 = ln(O)*g1+b1 + x
  FN = LN1 @ Wf^T + bf ; out = ln(FN)*g2+b2 + LN1

Restructuring (head h lives on core h):
  With xa = [x | 1] (augmented) and G~ = xa^T xa (symmetric; computed
  distributed over S and AllReduced in 3 row-chunks):
    scores_h = Wka_h G~ Wqa_h^T / sqrt(E)  where Wka = [Wk|bk], Wqa = [Wq|bq]
  A_h = softmax(scores_h). Then instead of the S-sized Q/K/Zh matmuls:
    O_h = V_h A_h Wz_h^T = x (Wv_h^T A_h Wz_h^T) + 1 (bv_h^T A_h Wz_h^T)
  so each core does ONE S-sized matmul: Opart = x @ C_h + 1 r_h^T, with
    AT = A^T (PE transpose), B[e,o] = sum_f A[e,f] WzT[f,o], C = Wv^T B,
    r = bv^T B + bz/8 (bz pre-scaled on host so the RS sum restores bz).
  Cross-core: chunked bf16 ReduceScatter(sum_h) of Opart shards S;
  LN1/FFN/LN2 pipeline per chunk on S/8 rows per core; the host
  reassembles the (chunk-interleaved) shards.
  A tiny AllReduce "barrier" runs first so the per-core start skew is
  absorbed before the latency-critical G AllReduce.
"""

import numpy as np
import ml_dtypes

import concourse.bass as bass
import concourse.mybir as mybir
import concourse.tile as tile
from concourse import bacc
from concourse.bass_utils import run_bass_kernel_spmd
from concourse.masks import make_identity

S, E, H = 4096, 1024, 8
P = 128
EA = 1152          # augmented (E + ones col) padded to 9*128
NET = E // P       # 8
NAT = EA // P      # 9
SS = S // H        # 512 rows per core after reduce-scatter
NST = SS // P      # 4
NMT = S // P       # 32
NH = E // 512      # 2 free-dim halves
NCH = 4            # reduce-scatter chunks (each SS/NCH = 128 rows/core)
MPC = NMT // NCH   # Opart m-tiles per RS chunk (8)
NGC = 3            # G AllReduce chunks (3 row-tiles each)
KPC = NAT // NGC   # k-tiles per G chunk
EPS = 1e-5
SCALE = 1.0 / 32.0  # 1/sqrt(E)

F32 = mybir.dt.float32
BF16 = mybir.dt.bfloat16

# packed rows input: [bz/8, g1, b1, bf, g2, b2]; rows_bc holds the last 5
L_G1, L_B1, L_BF, L_G2, L_B2 = range(5)

LAST_RESULT = None  # test harness reads exec_time_ns off this


def _bcast_row(t: bass.AP) -> bass.AP:
    """[1, n] DRAM row -> partition-broadcast AP."""
    return bass.AP(tensor=t.tensor, offset=t.offset, ap=[[0, P], [1, t.shape[-1]]])


def build_nc():
    nc = bacc.Bacc(num_devices=H)

    xt = nc.declare_dram_parameter("xt", [E, S], BF16, isOutput=False)
    xsa = nc.declare_dram_parameter("xsa", [SS, EA], BF16, isOutput=False)
    xs = nc.declare_dram_parameter("xs", [SS, E], F32, isOutput=False)
    wqa = nc.declare_dram_parameter("wqa", [EA, E], BF16, isOutput=False)
    wka = nc.declare_dram_parameter("wka", [EA, E], BF16, isOutput=False)
    wv = nc.declare_dram_parameter("wv", [E, E], BF16, isOutput=False)
    wzT = nc.declare_dram_parameter("wzT", [E, E], BF16, isOutput=False)
    wfT = nc.declare_dram_parameter("wfT", [E, E], BF16, isOutput=False)
    bv = nc.declare_dram_parameter("bv", [P, NET], BF16, isOutput=False)
    rows = nc.declare_dram_parameter("rows", [6, E], F32, isOutput=False)
    out = nc.declare_dram_parameter("out", [SS, E], F32, isOutput=True)

    bar_in = nc.dram_tensor("bar_in", [1, 512], F32)
    bar_out = nc.dram_tensor("bar_out", [1, 512], F32, addr_space="Shared")
    g_part = nc.dram_tensor("g_part", [EA, EA], BF16)
    g_full = nc.dram_tensor("g_full", [EA, EA], BF16, addr_space="Shared")
    r_dram = nc.dram_tensor("r_dram", [1, E], F32)
    op_bounce = nc.dram_tensor("op_bounce", [S, E], BF16)
    rs_out = nc.dram_tensor("rs_out", [SS, E], BF16)

    rg = [list(range(H))]

    def mm_loop(lhs_fn, rhs_fn, nk, evac, ps_pool):
        pss = [
            ps_pool.tile([P, 512], F32, tag="mm", name=f"psmm_{n}") for n in range(NH)
        ]
        for k in range(nk):
            lhs = lhs_fn(k)
            for n in range(NH):
                nc.tensor.matmul(
                    pss[n], lhs, rhs_fn(k, n), start=(k == 0), stop=(k == nk - 1)
                )
        for n in range(NH):
            evac(n, pss[n])

    with tile.TileContext(nc) as tc:
        with (
            tc.tile_pool(name="singles", bufs=1) as singles,
            tc.tile_pool(name="stat", bufs=4) as stat,
            tc.tile_pool(name="ps_mm", bufs=6, space="PSUM") as ps_mm,
            tc.tile_pool(name="ps_tr", bufs=2, space="PSUM") as ps_tr,
        ):
            ident = singles.tile([P, P], BF16)
            bz8_sb = singles.tile([1, E], F32)
            bv_sb = singles.tile([P, NET], BF16)
            rcp_sb = singles.tile([P, NET], F32)
            rbc_sb = singles.tile([P, E], F32)
            eps_sb = singles.tile([P, 1], F32)

            with tc.tile_pool(name="pc", bufs=1) as pc:
                c_sb = pc.tile([P, NET, E], BF16)
                with tc.tile_pool(name="pwz", bufs=1) as pwz:
                    wv_sb = pwz.tile([P, NET, E], BF16)
                    wzT_sb = pwz.tile([P, NET, E], BF16)
                    with tc.tile_pool(name="pb", bufs=1) as pb:
                        b_sb = pb.tile([P, NET, E], BF16)
                        with tc.tile_pool(name="pat", bufs=1) as pat:
                            at_sb = pat.tile([P, NET, E], BF16)
                            with tc.tile_pool(name="pwqk", bufs=1) as pwqk:
                                wqa_sb = pwqk.tile([P, NAT, E], BF16)
                                wka_sb = pwqk.tile([P, NAT, E], BF16)
                                u_sb = pwqk.tile([P, NAT, E], BF16)
                                with tc.tile_pool(name="pg", bufs=1) as pg, \
                                     tc.tile_pool(name="p1w", bufs=3) as p1w:
                                    # ===== phase 1: G~ partial + chunked AR =====
                                    xsa_sb = pg.tile([P, NST, EA], BF16)
                                    nc.sync.dma_start(
                                        out=xsa_sb,
                                        in_=xsa[:, :].rearrange("(t p) e -> p t e", p=P),
                                    )
                                    nchunks = [(0, 512), (512, 512), (1024, EA - 1024)]
                                    for ci in range(NGC):
                                        for mi in range(KPC):
                                            m = ci * KPC + mi
                                            gp = p1w.tile([P, EA], BF16, tag="gp")
                                            for (n0, nw) in nchunks:
                                                ps = ps_mm.tile(
                                                    [P, nw], F32, tag="mm", name="psg"
                                                )
                                                for k in range(NST):
                                                    nc.tensor.matmul(
                                                        ps,
                                                        xsa_sb[:, k, m * P : (m + 1) * P],
                                                        xsa_sb[:, k, n0 : n0 + nw],
                                                        start=(k == 0),
                                                        stop=(k == NST - 1),
                                                    )
                                                nc.vector.tensor_copy(
                                                    out=gp[:, n0 : n0 + nw], in_=ps
                                                )
                                            nc.sync.dma_start(
                                                out=g_part[m * P : (m + 1) * P, :], in_=gp
                                            )
                                        r0 = ci * KPC * P
                                        r1 = (ci + 1) * KPC * P
                                        nc.gpsimd.collective_compute(
                                            "AllReduce",
                                            mybir.AluOpType.add,
                                            replica_groups=rg,
                                            ins=[g_part[r0:r1, :]],
                                            outs=[g_full[r0:r1, :]],
                                        )

                                    # ---- constants / weights (emitted after
                                    # the collectives: G path wins DMA prio) ----
                                    make_identity(nc, ident)
                                    nc.sync.dma_start(out=bz8_sb, in_=rows[0:1, :])
                                    nc.sync.dma_start(out=bv_sb, in_=bv[:, :])
                                    nc.vector.memset(eps_sb, EPS)
                                    nc.sync.dma_start(
                                        out=wqa_sb,
                                        in_=wqa[:, :].rearrange("(t p) e -> p t e", p=P),
                                    )
                                    nc.sync.dma_start(
                                        out=wka_sb,
                                        in_=wka[:, :].rearrange("(t p) e -> p t e", p=P),
                                    )
                                    nc.sync.dma_start(
                                        out=wv_sb,
                                        in_=wv[:, :].rearrange("(t p) e -> p t e", p=P),
                                    )
                                    nc.sync.dma_start(
                                        out=wzT_sb,
                                        in_=wzT[:, :].rearrange("(t p) e -> p t e", p=P),
                                    )

                                    # ===== phase 2: U = G~ @ wqa, overlapping
                                    # the chunked AR (psum persists per chunk)
                                    g_sb = pg.tile([P, NAT, EA], BF16)
                                    for ci in range(NGC):
                                        nc.sync.dma_start(
                                            out=g_sb[:, ci * KPC : (ci + 1) * KPC, :],
                                            in_=g_full[
                                                ci * KPC * P : (ci + 1) * KPC * P, :
                                            ].rearrange("(t p) e -> p t e", p=P),
                                        )
                                    for (m0, m1) in [(0, 3), (3, 6), (6, 9)]:
                                        pss = {}
                                        for m in range(m0, m1):
                                            for n in range(NH):
                                                pss[m, n] = ps_mm.tile(
                                                    [P, 512], F32, tag="mm",
                                                    name=f"psu_{m}_{n}",
                                                )
                                        for ci in range(NGC):
                                            for m in range(m0, m1):
                                                for n in range(NH):
                                                    for kk in range(KPC):
                                                        k = ci * KPC + kk
                                                        nc.tensor.matmul(
                                                            pss[m, n],
                                                            g_sb[:, k, m * P : (m + 1) * P],
                                                            wqa_sb[:, k, n * 512 : (n + 1) * 512],
                                                            start=(k == 0),
                                                            stop=(k == NAT - 1),
                                                        )
                                        for m in range(m0, m1):
                                            for n in range(NH):
                                                nc.vector.tensor_copy(
                                                    out=u_sb[:, m, n * 512 : (n + 1) * 512],
                                                    in_=pss[m, n],
                                                )

                                # ===== phase 3: scores + softmax + A^T =====
                                with tc.tile_pool(name="p3", bufs=3) as p3:
                                    for m in range(NET):
                                        sc = p3.tile([P, E], F32, tag="sc")
                                        mm_loop(
                                            lambda k: wka_sb[:, k, m * P : (m + 1) * P],
                                            lambda k, n: u_sb[:, k, n * 512 : (n + 1) * 512],
                                            NAT,
                                            lambda n, ps: nc.vector.tensor_scalar_mul(
                                                sc[:, n * 512 : (n + 1) * 512], ps, SCALE
                                            ),
                                            ps_mm,
                                        )
                                        negmx = stat.tile([P, 1], F32, tag="negmx")
                                        nc.vector.reduce_max(
                                            out=negmx, in_=sc,
                                            axis=mybir.AxisListType.X, negate=True,
                                        )
                                        a_bf = p3.tile([P, E], BF16, tag="abf")
                                        rsum = stat.tile([P, 1], F32, tag="rsum")
                                        nc.scalar.activation(
                                            out=a_bf, in_=sc,
                                            func=mybir.ActivationFunctionType.Exp,
                                            bias=negmx, scale=1.0, accum_out=rsum,
                                        )
                                        nc.vector.reciprocal(
                                            out=rcp_sb[:, m : m + 1], in_=rsum
                                        )
                                        for fb in range(NET):
                                            pst = ps_tr.tile([P, P], BF16, tag="tr")
                                            nc.tensor.transpose(
                                                pst, a_bf[:, fb * P : (fb + 1) * P], ident
                                            )
                                            nc.vector.tensor_copy(
                                                out=at_sb[:, fb, m * P : (m + 1) * P],
                                                in_=pst,
                                            )

                            # ===== phase 4a: B = AT.T @ WzT (row-scaled) =====
                            for m in range(NET):
                                mm_loop(
                                    lambda k: at_sb[:, k, m * P : (m + 1) * P],
                                    lambda k, n: wzT_sb[:, k, n * 512 : (n + 1) * 512],
                                    NET,
                                    lambda n, ps: nc.vector.tensor_scalar_mul(
                                        b_sb[:, m, n * 512 : (n + 1) * 512],
                                        ps,
                                        rcp_sb[:, m : m + 1],
                                    ),
                                    ps_mm,
                                )

                        # ===== phase 4b: C = Wv^T B ; r = bv^T B + bz/8 =====
                        for m in range(NET):
                            mm_loop(
                                lambda k: wv_sb[:, k, m * P : (m + 1) * P],
                                lambda k, n: b_sb[:, k, n * 512 : (n + 1) * 512],
                                NET,
                                lambda n, ps: nc.vector.tensor_copy(
                                    out=c_sb[:, m, n * 512 : (n + 1) * 512], in_=ps
                                ),
                                ps_mm,
                            )
                        r_sb = stat.tile([1, E], F32, tag="rrow")
                        for n in range(NH):
                            psr = ps_mm.tile([1, 512], F32, tag="mm", name="psr")
                            for k in range(NET):
                                nc.tensor.matmul(
                                    psr,
                                    bv_sb[:, k : k + 1],
                                    b_sb[:, k, n * 512 : (n + 1) * 512],
                                    start=(k == 0),
                                    stop=(k == NET - 1),
                                )
                            nc.vector.tensor_add(
                                r_sb[:, n * 512 : (n + 1) * 512],
                                psr,
                                bz8_sb[:, n * 512 : (n + 1) * 512],
                            )
                        nc.sync.dma_start(out=r_dram[:, :], in_=r_sb)
                        nc.sync.dma_start(out=rbc_sb, in_=_bcast_row(r_dram[0:1, :]))

                # ===== phase 5: Opart + chunked RS; LN/FFN pipelined =====
                with tc.tile_pool(name="p5", bufs=3) as p5, \
                     tc.tile_pool(name="pln", bufs=1) as pln, \
                     tc.tile_pool(name="p7", bufs=2) as p7:
                    wfT_sb = pln.tile([P, NET, E], BF16)
                    nc.sync.dma_start(
                        out=wfT_sb, in_=wfT[:, :].rearrange("(t p) e -> p t e", p=P)
                    )
                    rows_bc = pln.tile([P, 5, E], F32)
                    for k in range(5):
                        nc.sync.dma_start(
                            out=rows_bc[:, k, :], in_=_bcast_row(rows[k + 1 : k + 2, :])
                        )
                    ln1_sb = pln.tile([P, NST, E], F32)
                    l1t_sb = pln.tile([P, NET, SS], BF16)
                    xt_re = xt[:, :].rearrange("(t p) s -> p t s", p=P)

                    def layer_norm(dst, src, r_g, r_b):
                        bst = stat.tile([P, 2, 6], F32, tag="bst")
                        nc.vector.bn_stats(out=bst[:, 0, :], in_=src[:, 0:512])
                        nc.vector.bn_stats(out=bst[:, 1, :], in_=src[:, 512:E])
                        mv = stat.tile([P, 2], F32, tag="mv")
                        nc.vector.bn_aggr(out=mv, in_=bst)
                        sd = stat.tile([P, 1], F32, tag="sd")
                        nc.scalar.activation(
                            out=sd, in_=mv[:, 1:2],
                            func=mybir.ActivationFunctionType.Sqrt, bias=eps_sb[:, :],
                        )
                        rstd = stat.tile([P, 1], F32, tag="rstd")
                        nc.vector.reciprocal(out=rstd, in_=sd)
                        nc.vector.tensor_scalar(
                            out=dst, in0=src, scalar1=mv[:, 0:1], scalar2=rstd,
                            op0=mybir.AluOpType.subtract, op1=mybir.AluOpType.mult,
                        )
                        nc.vector.tensor_mul(dst, dst, rows_bc[:, r_g, :])
                        nc.vector.tensor_add(dst, dst, rows_bc[:, r_b, :])

                    for c in range(NCH):
                        for mi in range(MPC):
                            m = c * MPC + mi
                            xtc = p5.tile([P, NET, P], BF16, tag="xtc", bufs=6)
                            nc.sync.dma_start(
                                out=xtc, in_=xt_re[:, :, m * P : (m + 1) * P]
                            )
                            o_sb = p5.tile([P, E], BF16, tag="osb")
                            mm_loop(
                                lambda k: xtc[:, k, :],
                                lambda k, n: c_sb[:, k, n * 512 : (n + 1) * 512],
                                NET,
                                lambda n, ps: nc.vector.tensor_add(
                                    o_sb[:, n * 512 : (n + 1) * 512],
                                    ps,
                                    rbc_sb[:, n * 512 : (n + 1) * 512],
                                ),
                                ps_mm,
                            )
                            nc.sync.dma_start(
                                out=op_bounce[m * P : (m + 1) * P, :], in_=o_sb
                            )
                        nc.gpsimd.collective_compute(
                            "ReduceScatter",
                            mybir.AluOpType.add,
                            replica_groups=rg,
                            ins=[op_bounce[c * MPC * P : (c + 1) * MPC * P, :]],
                            outs=[rs_out[c * P : (c + 1) * P, :]],
                        )

                    # LN1 + FFN + LN2 per chunk, after all Opart matmuls so
                    # the in-order PE/DVE queues never stall on an RS wait
                    for st in range(NCH):
                        t1 = ln1_sb[:, st, :]
                        ot = p7.tile([P, E], BF16, tag="ot")
                        nc.sync.dma_start(out=ot, in_=rs_out[st * P : (st + 1) * P, :])
                        ln = p7.tile([P, E], F32, tag="ln")
                        layer_norm(ln, ot, L_G1, L_B1)
                        xst = p7.tile([P, E], F32, tag="xst")
                        nc.sync.dma_start(out=xst, in_=xs[st * P : (st + 1) * P, :])
                        nc.vector.tensor_add(t1, ln, xst)
                        lbf = p7.tile([P, E], BF16, tag="lbf")
                        nc.vector.tensor_copy(out=lbf, in_=t1)
                        for eb in range(NET):
                            pst = ps_tr.tile([P, P], BF16, tag="tr")
                            nc.tensor.transpose(pst, lbf[:, eb * P : (eb + 1) * P], ident)
                            nc.vector.tensor_copy(
                                out=l1t_sb[:, eb, st * P : (st + 1) * P], in_=pst
                            )
                        f1 = p7.tile([P, E], F32, tag="f1")
                        mm_loop(
                            lambda k: l1t_sb[:, k, st * P : (st + 1) * P],
                            lambda k, n: wfT_sb[:, k, n * 512 : (n + 1) * 512],
                            NET,
                            lambda n, ps: nc.vector.tensor_add(
                                f1[:, n * 512 : (n + 1) * 512],
                                ps,
                                rows_bc[:, L_BF, n * 512 : (n + 1) * 512],
                            ),
                            ps_mm,
                        )
                        ln2 = p7.tile([P, E], F32, tag="ln2")
                        layer_norm(ln2, f1, L_G2, L_B2)
                        fo = p7.tile([P, E], F32, tag="ln")
                        nc.vector.tensor_add(fo, ln2, ln1_sb[:, st, :])
                        nc.sync.dma_start(out=out[st * P : (st + 1) * P, :], in_=fo)

    nc.finalize()
    return nc


_NC_CACHE = None


def _shard_rows(h):
    """Global S-rows owned by core h (RS chunk layout)."""
    idx = []
    for c in range(NCH):
        base = c * (S // NCH) + h * P
        idx.extend(range(base, base + P))
    return np.array(idx)


def kernel(**inputs) -> np.ndarray:
    global _NC_CACHE, LAST_RESULT
    x = np.asarray(inputs["x"], np.float32)
    Wq = np.asarray(inputs["Wq"], np.float32)
    bq = np.asarray(inputs["bq"], np.float32)
    Wk = np.asarray(inputs["Wk"], np.float32)
    bk = np.asarray(inputs["bk"], np.float32)
    Wv = np.asarray(inputs["Wv"], np.float32)
    bv = np.asarray(inputs["bv"], np.float32)
    Wz = np.asarray(inputs["Wz"], np.float32)
    bz = np.asarray(inputs["bz"], np.float32)
    g1 = np.asarray(inputs["g1"], np.float32)
    b1 = np.asarray(inputs["b1"], np.float32)
    Wf = np.asarray(inputs["Wf"], np.float32)
    bf_ = np.asarray(inputs["bf"], np.float32)
    g2 = np.asarray(inputs["g2"], np.float32)
    b2 = np.asarray(inputs["b2"], np.float32)

    BF = ml_dtypes.bfloat16
    if _NC_CACHE is None:
        _NC_CACHE = build_nc()
    nc = _NC_CACHE

    xt_np = np.ascontiguousarray(x.T).astype(BF)
    wfT_np = np.ascontiguousarray(Wf.T).astype(BF)
    rows_np = np.ascontiguousarray(
        np.stack([bz / H, g1, b1, bf_, g2, b2]).astype(np.float32)
    )
    pad_w = np.zeros((EA - E - 1, E), np.float32)

    in_maps = []
    for h in range(H):
        gsl = slice(h * SS, (h + 1) * SS)  # contiguous shard for G partial
        xga = x[gsl]
        xsa_h = np.concatenate(
            [xga, np.ones((SS, 1), np.float32), np.zeros((SS, EA - E - 1), np.float32)],
            axis=1,
        ).astype(BF)
        xs_h = np.ascontiguousarray(x[_shard_rows(h)])  # residual rows (RS layout)
        wqa_h = np.concatenate([Wq[h].T, bq[h][None, :], pad_w], axis=0).astype(BF)
        wka_h = np.concatenate([Wk[h].T, bk[h][None, :], pad_w], axis=0).astype(BF)
        wzT_h = np.ascontiguousarray(Wz[:, h * E : (h + 1) * E].T).astype(BF)
        bv_h = np.ascontiguousarray(bv[h].reshape(NET, P).T).astype(BF)
        in_maps.append(
            {
                "xt": xt_np,
                "xsa": np.ascontiguousarray(xsa_h),
                "xs": xs_h,
                "wqa": np.ascontiguousarray(wqa_h),
                "wka": np.ascontiguousarray(wka_h),
                "wv": Wv[h].astype(BF),
                "wzT": wzT_h,
                "wfT": wfT_np,
                "bv": bv_h,
                "rows": rows_np,
            }
        )

    res = run_bass_kernel_spmd(nc, in_maps, list(range(H)))
    LAST_RESULT = res
    out = np.empty((S, E), np.float32)
    for h in range(H):
        out[_shard_rows(h)] = res.results[h]["out"]
    return out
